# revision 1
# baseline (speedup 1.0000x reference)
"""Trainium2 Bass kernel for the combined Tversky + Focal + Boundary loss.

Strategy (8 NeuronCores, SPMD single program, per-core data differs):
  - Core pair (2m, 2m+1) handles batch element b=m; even core owns the
    d-half [0,32), odd core [32,64) of all of b's EDT volumes.
  - Part A (dice + focal): every core processes the D-slab d in [8k, 8k+8)
    of ALL batch elements (union of 8 slabs = full volume), producing
    per-(b,c) partials of TP, sum(p), count, focal via fused accum_out.
    Partition dim = (b, spatial/32) so per-partition accums are per-b.
  - Part B (boundary): per core, 6 half-volume EDT tasks = 3 slots
    (slot s = class s+1, partitions 0-63 'out' polarity / 64-127 'in').
    Exact truncated min-plus EDT with radius R (validated host-side, R=3):
    pass W then D in layout [h][(slot,d,w)], PE transpose to [w][(slot,d,h)],
    pass H; the d halo (R rows) is baked into the host-built seed masks so
    the program is parity-agnostic.  sqrt * softmax-prob summed via fused
    scalar_tensor_tensor accum.
  - Host gathers the tiny per-core stats and assembles the scalar loss.
"""

import sys

for _p in ("/opt/trn_rl_repo",):
    if _p not in sys.path:
        sys.path.insert(0, _p)

import numpy as np
import ml_dtypes

NUM_CLASSES = 4
N = 64
V = N * N * N
B = 4
HALF = 32          # d-rows per core
T_ALPHA, T_BETA = 0.3, 0.7
SMOOTH = 1e-5
W_DICE, W_CE, W_BOUND = 1.0, 1.0, 0.01
INF = 16384.0
CLS_ORDER = (1, 2, 3, 0)   # pvol class order: slot s uses pos s (classes 1..3)

_PROGRAM_CACHE = {}


def _compute_R(targets):
    """Smallest R such that the per-axis-truncated 3-pass EDT is exact for
    all 24 seed sets (masks and their complements)."""
    seeds = []
    for b in range(B):
        for c in range(1, NUM_CLASSES):
            m = targets[b] == c
            s = int(m.sum())
            if 0 < s < m.size:
                seeds.append(m)
                seeds.append(~m)
    if not seeds:
        return 1
    stack = np.stack(seeds)
    for Rc in (3, 4, 5, 6, 8, 10, 13, 16, 21, 27, 34, 43, 54, 63):
        f = np.where(stack, np.float32(0.0), np.float32(1e9))
        for ax in (1, 2, 3):
            fm = np.moveaxis(f, ax, -1)
            acc = fm.copy()
            for d in range(1, Rc + 1):
                d2 = np.float32(d * d)
                np.minimum(acc[..., :-d], fm[..., d:] + d2, out=acc[..., :-d])
                np.minimum(acc[..., d:], fm[..., :-d] + d2, out=acc[..., d:])
            f = np.moveaxis(acc, -1, ax)
        if f.max() <= Rc * Rc:
            return Rc
    return 63


def _build_program(R, reps=1):
    import concourse.bacc as bacc
    import concourse.tile as tile
    from concourse import mybir

    AF = mybir.ActivationFunctionType
    Alu = mybir.AluOpType
    f32 = mybir.dt.float32
    bf16 = mybir.dt.bfloat16
    u8 = mybir.dt.uint8

    E = HALF + 2 * R          # extended d rows
    WP = N + 2 * R            # padded w (and h) length
    HV = HALF * N             # half-volume free size per w-partition (2048)
    S = NUM_CLASSES - 1       # 3 slots

    nc = bacc.Bacc("TRN2", target_bir_lowering=False, debug=False, num_devices=8)

    slabA = nc.declare_dram_parameter("slabA", [128, NUM_CLASSES, 1024], f32, isOutput=False)
    targA = nc.declare_dram_parameter("targA", [128, 1024], u8, isOutput=False)
    maskI = nc.declare_dram_parameter("maskI", [128, S, E, N], u8, isOutput=False)
    pvol = nc.declare_dram_parameter("pvol", [128, NUM_CLASSES, HV], f32, isOutput=False)
    ident = nc.declare_dram_parameter("ident", [128, 64], bf16, isOutput=False)
    stats_out = nc.declare_dram_parameter("stats", [128, 64], f32, isOutput=True)

    deltas = [sg * r for r in range(1, R + 1) for sg in (1, -1)]

    with tile.TileContext(nc) as tc:
        with (
            tc.tile_pool(name="glob", bufs=1) as glob,
            tc.tile_pool(name="pa", bufs=1) as pa,
            tc.tile_pool(name="paw", bufs=2) as paw,
            tc.tile_pool(name="paw1", bufs=1) as paw1,
            tc.tile_pool(name="pb", bufs=1) as pb,
            tc.tile_pool(name="pbw", bufs=2) as pbw,
            tc.tile_pool(name="pbw1", bufs=1) as pbw1,
            tc.tile_pool(name="pbw3", bufs=3) as pbw3,
            tc.tile_pool(name="psum", bufs=2, space="PSUM") as psum_pool,
        ):
            stats = glob.tile([128, 64], f32)
            nc.gpsimd.memset(stats, 0.0)
            identt = glob.tile([128, 64], bf16)
            nc.sync.dma_start(out=identt, in_=ident[:])

            src0 = pb.tile([128, S, E, WP], bf16)
            acc1 = pb.tile([128, S, E, N], bf16)
            acc2 = pb.tile([128, S, HALF, N], bf16)
            L2 = pb.tile([128, S, HALF, WP], bf16)
            acc3 = pb.tile([128, S, HALF, N], bf16, tag="acc1" if reps == 1 else "acc3")
            g = pb.tile([128, S, HV], bf16)
            tmp = pb.tile([128, S, E, N], bf16)
            nc.gpsimd.memset(src0, INF)
            nc.gpsimd.memset(L2, INF)

            for _rep in range(reps):
                # ================= Part A: dice + focal on the D-slab =========
                targ = pa.tile([128, 1024], u8)
                nc.sync.dma_start(out=targ, in_=targA[:])
                ec = []
                for c in range(NUM_CLASSES):
                    sl = paw.tile([128, 1024], f32, tag="sl")
                    nc.sync.dma_start(out=sl, in_=slabA[:, c, :])
                    e = pa.tile([128, 1024], f32, tag=f"ec{c}")
                    nc.scalar.activation(out=e, in_=sl, func=AF.Exp)
                    ec.append(e)
                se = pa.tile([128, 1024], f32)
                nc.vector.tensor_add(se, ec[0], ec[1])
                nc.vector.tensor_add(se, se, ec[2])
                nc.vector.tensor_add(se, se, ec[3])
                rse = pa.tile([128, 1024], f32)
                nc.vector.reciprocal(rse, se)

                pt = pa.tile([128, 1024], f32)
                for c in range(NUM_CLASSES):
                    mc = paw1.tile([128, 1024], f32, tag="mc")
                    nc.vector.tensor_scalar(mc, targ, float(c), 0.0, Alu.is_equal,
                                            Alu.add, accum_out=stats[:, 8 + c:9 + c])
                    pc = paw1.tile([128, 1024], f32, tag="pc")
                    nc.vector.scalar_tensor_tensor(
                        out=pc, in0=ec[c], scalar=1.0, in1=rse,
                        op0=Alu.mult, op1=Alu.mult,
                        accum_out=stats[:, 4 + c:5 + c])
                    prod = paw1.tile([128, 1024], f32, tag="prod")
                    nc.vector.scalar_tensor_tensor(
                        out=prod, in0=pc, scalar=1.0, in1=mc,
                        op0=Alu.mult, op1=Alu.mult,
                        accum_out=stats[:, 0 + c:1 + c])
                    if c == 0:
                        nc.vector.tensor_copy(pt, prod)
                    else:
                        nc.vector.tensor_add(pt, pt, prod)

                w2 = paw1.tile([128, 1024], f32, tag="mc")
                nc.vector.tensor_scalar(w2, pt, -1.0, 1.0, Alu.mult, Alu.add)
                w2sq = paw1.tile([128, 1024], f32, tag="pc")
                nc.scalar.activation(out=w2sq, in_=w2, func=AF.Square)
                lpt = paw1.tile([128, 1024], f32, tag="prod")
                nc.scalar.activation(out=lpt, in_=pt, func=AF.Ln)
                fsc = paw1.tile([128, 1024], f32, tag="fsc")
                nc.vector.scalar_tensor_tensor(
                    out=fsc, in0=w2sq, scalar=1.0, in1=lpt,
                    op0=Alu.mult, op1=Alu.mult,
                    accum_out=stats[:, 12:13])

                # ================= Part B: boundary ===========================
                # softmax denominator on the half-volume (w-partitions, both
                # halves duplicated); keep exp of classes 1..3 for the slots.
                epos = []
                se2 = pb.tile([128, HV], f32)
                for pos in range(NUM_CLASSES):
                    pcls = pbw.tile([128, HV], f32, tag="pcls")
                    nc.sync.dma_start(out=pcls, in_=pvol[:, pos, :])
                    if pos < S:
                        et = pb.tile([128, HV], bf16, tag=f"epos{pos}")
                        epos.append(et)
                    else:
                        et = pbw1.tile([128, HV], f32, tag="etmp")
                    nc.scalar.activation(out=et, in_=pcls, func=AF.Exp)
                    if pos == 1:
                        nc.vector.tensor_add(se2, epos[0], epos[1])
                    elif pos > 1:
                        nc.vector.tensor_add(se2, se2, et)
                rse2 = pb.tile([128, HV], f32)
                nc.vector.reciprocal(rse2, se2)

                # masks -> src0 (INF where no seed)
                if _rep == 0:
                    for s in range(S):
                        maskt = pbw3.tile([128, E, N], u8, tag="maskt")
                        nc.sync.dma_start(out=maskt, in_=maskI[:, s, :, :])
                        nc.vector.tensor_scalar(
                            src0[:, s, :, R:R + N], maskt, -INF, INF, Alu.mult, Alu.add)

                # pass W (all slots, all extended rows)
                for r in range(1, R + 1):
                    tw = tmp[:, :, :, :]
                    nc.vector.tensor_tensor(
                        tw, src0[:, :, :, R + r:R + r + N],
                        src0[:, :, :, R - r:R - r + N], op=Alu.min)
                    nc.vector.tensor_scalar(tw, tw, float(r * r), None, Alu.add)
                    nc.vector.tensor_tensor(
                        acc1, src0[:, :, :, R:R + N] if r == 1 else acc1,
                        tw, op=Alu.min)
                # pass D
                for r in range(1, R + 1):
                    td = tmp[:, :, :HALF, :]
                    nc.vector.tensor_tensor(
                        td, acc1[:, :, R + r:R + r + HALF, :],
                        acc1[:, :, R - r:R - r + HALF, :], op=Alu.min)
                    nc.vector.tensor_scalar(td, td, float(r * r), None, Alu.add)
                    nc.vector.tensor_tensor(
                        acc2, acc1[:, :, R:R + HALF, :] if r == 1 else acc2,
                        td, op=Alu.min)
                # transpose [h][w] -> [w][h] per (slot, d) plane via PE
                for s in range(S):
                    for chunk in range(HALF // 8):
                        pst = psum_pool.tile([128, 8, 64], bf16, tag=f"pst{s}")
                        for dd in range(8):
                            dp = chunk * 8 + dd
                            nc.tensor.transpose(
                                pst[0:64, dd, :], acc2[0:64, s, dp, :],
                                identt[0:64, :])
                            nc.tensor.transpose(
                                pst[64:128, dd, :], acc2[64:128, s, dp, :],
                                identt[64:128, :])
                        nc.vector.tensor_copy(
                            out=L2[:, s, chunk * 8:chunk * 8 + 8, R:R + N],
                            in_=pst[:])
                # pass H
                for r in range(1, R + 1):
                    th = tmp[:, :, :HALF, :]
                    nc.vector.tensor_tensor(
                        th, L2[:, :, :, R + r:R + r + N],
                        L2[:, :, :, R - r:R - r + N], op=Alu.min)
                    nc.vector.tensor_scalar(th, th, float(r * r), None, Alu.add)
                    nc.vector.tensor_tensor(
                        acc3, L2[:, :, :, R:R + N] if r == 1 else acc3,
                        th, op=Alu.min)
                # g = sqrt(dist^2), then per-slot weighted sums
                nc.scalar.activation(out=g, in_=acc3, func=AF.Sqrt)
                for s in range(S):
                    uu = pbw1.tile([128, HV], f32, tag="uu")
                    nc.vector.tensor_mul(uu, epos[s], rse2)
                    nc.vector.scalar_tensor_tensor(
                        out=uu, in0=g[:, s, :], scalar=1.0, in1=uu,
                        op0=Alu.mult, op1=Alu.mult,
                        accum_out=stats[:, 16 + s:17 + s])

            nc.sync.dma_start(out=stats_out[:], in_=stats)

    nc.compile()
    return nc


def _core_inputs(k, preds, targets_u8, R):
    b, parity = k // 2, k % 2
    d0slab = 8 * k
    d0 = HALF * parity
    E = HALF + 2 * R
    S = NUM_CLASSES - 1

    slabA = np.ascontiguousarray(
        preds[:, :, d0slab:d0slab + 8].reshape(B, NUM_CLASSES, 32, 1024)
        .transpose(0, 2, 1, 3).reshape(128, NUM_CLASSES, 1024))
    targA = targets_u8[:, d0slab:d0slab + 8].reshape(128, 1024).copy()

    # masks with d halo baked in (zeros outside the volume)
    mk = np.zeros((2, S, E, N, N), np.uint8)  # [pol, slot, q, h, w]
    for s in range(S):
        c = s + 1
        m = (targets_u8[b] == c)
        for pol in range(2):
            seed = m if pol == 0 else (1 - m)
            lo = d0 - R
            for q in range(E):
                d = lo + q
                if 0 <= d < N:
                    mk[pol, s, q] = seed[d]
    # [pol, slot, q, h, w] -> partitions (pol, h), free (slot, q, w)
    maskIA = np.ascontiguousarray(
        mk.transpose(0, 3, 1, 2, 4).reshape(128, S, E, N))

    x = preds[b][list(CLS_ORDER)][:, d0:d0 + HALF]          # [4, 32, 64(h), 64(w)]
    x = np.ascontiguousarray(x.transpose(3, 0, 1, 2)).reshape(64, NUM_CLASSES, HALF * N)
    pvolA = np.ascontiguousarray(np.concatenate([x, x], axis=0))

    ident64 = np.eye(64, dtype=ml_dtypes.bfloat16)
    identA = np.concatenate([ident64, ident64], axis=0)

    return {"slabA": slabA, "targA": targA, "maskI": maskIA,
            "pvol": pvolA, "ident": identA}


def _assemble(results, targets_u8):
    TP = np.zeros((NUM_CLASSES, B), np.float64)
    Sp = np.zeros((NUM_CLASSES, B), np.float64)
    cnt = np.zeros((NUM_CLASSES, B), np.float64)
    focal_sum = 0.0
    S = np.zeros((B, NUM_CLASSES, 2), np.float64)  # [b, class, pol]

    for k in range(8):
        st = results[k]["stats"].astype(np.float64)
        b = k // 2
        for c in range(NUM_CLASSES):
            for bb in range(B):
                rows = slice(32 * bb, 32 * bb + 32)
                TP[c, bb] += st[rows, 0 + c].sum()
                Sp[c, bb] += st[rows, 4 + c].sum()
                cnt[c, bb] += st[rows, 8 + c].sum()
        focal_sum += st[:, 12].sum()
        for s in range(NUM_CLASSES - 1):
            S[b, s + 1, 0] += st[0:64, 16 + s].sum()
            S[b, s + 1, 1] += st[64:128, 16 + s].sum()

    tv = np.zeros((NUM_CLASSES, B), np.float64)
    for c in range(NUM_CLASSES):
        for b in range(B):
            tp = TP[c, b]
            fp = Sp[c, b] - tp
            fn = cnt[c, b] - tp
            tv[c, b] = (tp + SMOOTH) / (tp + T_ALPHA * fp + T_BETA * fn + SMOOTH)
    l_dice = 1.0 - tv.mean()
    l_main = -focal_sum / (B * V)

    bsum = 0.0
    for b in range(B):
        for c in range(1, NUM_CLASSES):
            n_bc = cnt[c, b]
            if n_bc <= 0:
                continue
            if n_bc >= V:
                contrib = -Sp[c, b] / V
            else:
                contrib = (S[b, c, 0] - S[b, c, 1]) / V
            bsum += contrib
    l_bound = bsum / (B * (NUM_CLASSES - 1) + 1e-8)

    return np.float32(W_DICE * l_dice + W_CE * l_main + W_BOUND * l_bound)


def run(preds, targets, trace=False, trace_kwargs=None):
    from concourse.bass_utils import run_bass_kernel_spmd

    preds = np.asarray(preds, dtype=np.float32)
    targets_u8 = np.asarray(targets).astype(np.uint8)

    R = _compute_R(targets_u8)
    if R not in _PROGRAM_CACHE:
        _PROGRAM_CACHE[R] = _build_program(R)
    nc = _PROGRAM_CACHE[R]

    in_maps = [_core_inputs(k, preds, targets_u8, R) for k in range(8)]
    kw = dict(trace=trace)
    if trace_kwargs:
        kw.update(trace_kwargs)
    res = run_bass_kernel_spmd(nc, in_maps, list(range(8)), **kw)
    out = _assemble(res.results, targets_u8)
    return out, res


def kernel(preds, targets):
    out, _ = run(preds, targets, trace=False)
    return out

